# revision 8
# baseline (speedup 1.0000x reference)
"""Cross-attention kernel for Trainium2, 8-core SPMD.

Reference computation (per nn_CrossAttention):
    qh  = (q @ Wq)            -> heads           (B, NQ, H, DH)
    k,v = split(kv @ Wkv)     -> heads           (B, NKV, H, DH)
    out = softmax(qh @ kh^T * DH^-0.5) @ vh      (B, NQ, H, DH)
    out = out @ Wfc + bfc ; out = 2*out ; LayerNorm(eps=1e-6)*gamma+beta

Sharding: core c <- (batch b = c//2, query-row half r = c%2).
Each core processes 512 query rows with the full 1024-row kv of its batch
(kv projection duplicated between the two cores of a batch; weights
replicated).  No collectives.

Per-core kernel keeps activations feature-major ([feature, token]) so the
natural [in, out] weight layout is always the matmul lhsT and no on-chip
transposes are needed; the host passes q and kv pre-transposed.  fp32
storage everywhere with fp32r matmuls (full PE rate at N=512).
"""

import numpy as np

B, NQ, NKV = 4, 1024, 1024
D = 512          # model dim == DQ == DKV == INNER
H, DH = 8, 64
P = 128
ROWS = NQ // 2   # query rows per core
EPS = 1e-6
SCALE = DH ** -0.5

_CACHE = {}


def _build_program():
    from concourse import bacc, tile
    from concourse.tile import add_dep_helper
    import concourse.bass as bass
    import concourse.mybir as mybir

    f32 = mybir.dt.float32
    bf16 = mybir.dt.bfloat16
    f32r = mybir.dt.float32r
    ALU = mybir.AluOpType
    AF = mybir.ActivationFunctionType
    AX = mybir.AxisListType

    nc = bacc.Bacc("TRN2", target_bir_lowering=False, debug=False, num_devices=8)

    qT = nc.dram_tensor("qT", [D, ROWS], f32r, kind="ExternalInput").ap()
    kvT = nc.dram_tensor("kvT", [D, NKV], f32r, kind="ExternalInput").ap()
    Wq = nc.dram_tensor("Wq", [D, D], f32r, kind="ExternalInput").ap()
    Wk = nc.dram_tensor("Wk", [D, D], f32r, kind="ExternalInput").ap()
    Wv = nc.dram_tensor("Wv", [D, D], f32r, kind="ExternalInput").ap()
    Wfc = nc.dram_tensor("Wfc", [D, D], f32r, kind="ExternalInput").ap()
    bfc = nc.dram_tensor("bfc", [D], f32, kind="ExternalInput").ap()
    gamma = nc.dram_tensor("gamma", [D], f32, kind="ExternalInput").ap()
    beta = nc.dram_tensor("beta", [D], f32, kind="ExternalInput").ap()
    out = nc.dram_tensor("out", [ROWS, D], f32, kind="ExternalOutput").ap()

    KO = D // P           # 4 contraction chunks of 128
    NJ = NKV // P         # 8 key-row chunks of 128

    def r3(ap, n):
        # [ (ko p), n ] DRAM view -> [p, ko, n]
        return ap.rearrange("(ko p) n -> p ko n", p=P)

    with tile.TileContext(nc) as tc:
        with (
            tc.tile_pool(name="weights", bufs=1) as wp,
            tc.tile_pool(name="acts", bufs=1) as ap_pool,
            tc.tile_pool(name="exp", bufs=6) as ep,
            tc.tile_pool(name="rec", bufs=2) as rp,
            tc.tile_pool(name="ln", bufs=3) as lp,
            tc.tile_pool(name="psum_pj", bufs=2, space="PSUM") as pp,
            tc.tile_pool(name="psum_sim", bufs=2, space="PSUM") as ps,
            tc.tile_pool(name="psum_av", bufs=2, space="PSUM") as pa,
        ):
            # ---- load weights + inputs (feature-major) -------------------
            wq_sb = wp.tile([P, KO, D], f32r, tag="wq")
            nc.sync.dma_start(wq_sb[:], r3(Wq, D))
            qT_sb = ap_pool.tile([P, KO, ROWS], f32r, tag="qt")
            nc.sync.dma_start(qT_sb[:], r3(qT, ROWS))
            kvT_sb = ap_pool.tile([P, KO, NKV], f32r, tag="kvt")
            nc.sync.dma_start(kvT_sb[:], r3(kvT, NKV))
            wk_sb = wp.tile([P, KO, D], f32r, tag="wk")
            nc.sync.dma_start(wk_sb[:], r3(Wk, D))
            wv_sb = wp.tile([P, KO, D], f32r, tag="wv")
            nc.sync.dma_start(wv_sb[:], r3(Wv, D))
            wfc_sb = wp.tile([P, KO, D], f32r, tag="wfc")
            nc.sync.dma_start(wfc_sb[:], r3(Wfc, D))

            # per-channel LN params replicated across partitions via DMA
            bfc_rep = wp.tile([P, D], f32, tag="bfc")
            nc.sync.dma_start(bfc_rep[:], bfc[None, :].to_broadcast((P, D)))
            gamma_rep = wp.tile([P, D], f32, tag="gamma")
            nc.sync.dma_start(gamma_rep[:], gamma[None, :].to_broadcast((P, D)))
            beta_rep = wp.tile([P, D], f32, tag="beta")
            nc.sync.dma_start(beta_rep[:], beta[None, :].to_broadcast((P, D)))

            ones_sb = wp.tile([P, DH], bf16, tag="ones")
            nc.vector.memset(ones_sb[:], 1.0)
            eps_sb = wp.tile([P, 1], f32, tag="eps")
            nc.vector.memset(eps_sb[:], EPS)
            # 2*bfc, used when fusing the fc bias with the 'out += out'
            bfc2_rep = wp.tile([P, D], f32, tag="bfc2")
            nc.vector.tensor_scalar_mul(bfc2_rep[:], bfc_rep[:], 2.0)

            # ---- projections (feature-major outputs) ---------------------
            # qhT[d, i] : heads 2m / 2m+1 live on partitions 0-63 / 64-127
            qhT_sb = ap_pool.tile([P, KO, ROWS], f32r, tag="qht")
            for m in range(KO):
                pt = pp.tile([P, ROWS], f32, tag="pj")
                for ko in range(KO):
                    nc.tensor.matmul(
                        pt[:],
                        lhsT=wq_sb[:, ko, m * P:(m + 1) * P],
                        rhs=qT_sb[:, ko, :],
                        start=(ko == 0), stop=(ko == KO - 1),
                    )
                nc.vector.tensor_copy(qhT_sb[:, m, :], pt[:])

            # khT[d, j]
            khT_sb = ap_pool.tile([P, KO, NKV], f32r, tag="kht")
            for m in range(KO):
                for jh in range(2):
                    pt = pp.tile([P, ROWS], f32, tag="pj")
                    for ko in range(KO):
                        nc.tensor.matmul(
                            pt[:],
                            lhsT=wk_sb[:, ko, m * P:(m + 1) * P],
                            rhs=kvT_sb[:, ko, jh * ROWS:(jh + 1) * ROWS],
                            start=(ko == 0), stop=(ko == KO - 1),
                        )
                    nc.vector.tensor_copy(khT_sb[:, m, jh * ROWS:(jh + 1) * ROWS], pt[:])

            # vh[j, d] (token-major — consumed as AV lhsT)
            vh_sb = ap_pool.tile([P, NJ, D], bf16, tag="vh")
            for jc in range(NJ):
                pt = pp.tile([P, ROWS], f32, tag="pj")
                for ko in range(KO):
                    nc.tensor.matmul(
                        pt[:],
                        lhsT=kvT_sb[:, ko, jc * P:(jc + 1) * P],
                        rhs=wv_sb[:, ko, :],
                        start=(ko == 0), stop=(ko == KO - 1),
                    )
                nc.vector.tensor_copy(vh_sb[:, jc, :], pt[:])

            # ---- attention, one head pair (2m, 2m+1) at a time -----------
            innerT_sb = ap_pool.tile([P, KO, ROWS], f32r, tag="innerT")
            for m in range(KO):
                lo = slice(0, DH)        # head 2m     on partitions 0-63
                hi = slice(DH, P)        # head 2m+1   on partitions 64-127
                e_tiles = []
                for jc in range(NJ):
                    sp = ps.tile([P, 2, ROWS], f32, tag="sim")
                    # simT[j, i] = sum_d khT[d, j] * qhT[d, i]; the two heads
                    # use disjoint PE row groups -> run concurrently
                    nc.tensor.matmul(
                        sp[:, 0, :],
                        lhsT=khT_sb[lo, m, jc * P:(jc + 1) * P],
                        rhs=qhT_sb[lo, m, :],
                        start=True, stop=True,
                    )
                    nc.tensor.matmul(
                        sp[:, 1, :],
                        lhsT=khT_sb[hi, m, jc * P:(jc + 1) * P],
                        rhs=qhT_sb[hi, m, :],
                        start=True, stop=True,
                    )
                    et = ep.tile([P, 2, ROWS], bf16, tag="exp")
                    # softmax numerator; |SCALE*sim| <~ 1.2 so no max-shift
                    nc.scalar.activation(et[:], sp[:], AF.Exp, scale=SCALE)
                    e_tiles.append(et)

                av = pa.tile([P, ROWS], f32, tag="av")    # rows 0-63: head 2m
                sums = pa.tile([P, ROWS], f32, tag="av")  # row-sums, replicated
                prev_av = prev_sums = None
                for jc in range(NJ):
                    et = e_tiles[jc]
                    # Each bank holds ONE accumulation group spanning both
                    # heads' chains: start only on the first matmul issued
                    # into the bank, stop on the last.  Per-element
                    # pending-zero makes each chain's first write an
                    # overwrite and later writes accumulate.  The two
                    # chains touch disjoint partition slices, so Tile sees
                    # no WAW hazard — chain explicit ordering deps to keep
                    # the start-matmul first and stop-matmul last.
                    st = (jc == 0)
                    sto = (jc == NJ - 1)
                    # head 2m -> PSUM partitions 0-63 (PE col groups 0-1)
                    a0 = nc.tensor.matmul(
                        av[:DH, :],
                        lhsT=vh_sb[:, jc, 2 * m * DH:(2 * m + 1) * DH],
                        rhs=et[:, 0, :],
                        start=st, stop=sto, skip_group_check=True,
                    )
                    # head 2m+1 -> PSUM partitions 64-127 (col groups 2-3)
                    a1 = nc.tensor.matmul(
                        av[DH:, :],
                        lhsT=vh_sb[:, jc, (2 * m + 1) * DH:(2 * m + 2) * DH],
                        rhs=et[:, 1, :],
                        start=st, stop=sto, skip_group_check=True,
                        tile_position=(0, 64),
                    )
                    s0 = nc.tensor.matmul(
                        sums[:DH, :],
                        lhsT=ones_sb[:],
                        rhs=et[:, 0, :],
                        start=st, stop=sto, skip_group_check=True,
                    )
                    s1 = nc.tensor.matmul(
                        sums[DH:, :],
                        lhsT=ones_sb[:],
                        rhs=et[:, 1, :],
                        start=st, stop=sto, skip_group_check=True,
                        tile_position=(0, 64),
                    )
                    if prev_av is not None:
                        add_dep_helper(a0.ins, prev_av.ins, sync=False,
                                       reason="av group order")
                        add_dep_helper(s0.ins, prev_sums.ins, sync=False,
                                       reason="sums group order")
                    add_dep_helper(a1.ins, a0.ins, sync=False,
                                   reason="av group order")
                    add_dep_helper(s1.ins, s0.ins, sync=False,
                                   reason="sums group order")
                    prev_av, prev_sums = a1, s1
                rec = rp.tile([P, ROWS], f32, tag="rec")
                nc.vector.reciprocal_approx_fast(rec[:], sums[:])
                nc.vector.tensor_tensor(innerT_sb[:, m, :], av[:], rec[:], ALU.mult)

            # ---- fc + LayerNorm ------------------------------------------
            out_r = out.rearrange("(ic p) n -> p ic n", p=P)
            for ic in range(KO):
                pt = pp.tile([P, ROWS], f32, tag="pj")
                for ko in range(KO):
                    nc.tensor.matmul(
                        pt[:],
                        lhsT=innerT_sb[:, ko, ic * P:(ic + 1) * P],
                        rhs=wfc_sb[:, ko, :],
                        start=(ko == 0), stop=(ko == KO - 1),
                    )
                # z = 2*(fc + bfc)   (the reference's `out += out`)
                z = lp.tile([P, D], f32, tag="z")
                nc.vector.scalar_tensor_tensor(
                    z[:], pt[:], 2.0, bfc2_rep[:], ALU.mult, ALU.add
                )
                st6 = lp.tile([P, 6], f32, tag="st6")
                nc.vector.bn_stats(st6[:], z[:])
                mv = lp.tile([P, 2], f32, tag="mv")
                nc.vector.bn_aggr(mv[:], st6[:])
                std = lp.tile([P, 1], f32, tag="std")
                nc.scalar.activation(std[:], mv[:, 1:2], AF.Sqrt, bias=eps_sb[:])
                rstd = lp.tile([P, 1], f32, tag="rstd")
                nc.vector.reciprocal(rstd[:], std[:])
                negm2 = lp.tile([P, 1], f32, tag="negm2")
                nc.vector.scalar_tensor_tensor(
                    negm2[:], mv[:, 0:1], -1.0, rstd[:], ALU.mult, ALU.mult
                )
                y = lp.tile([P, D], f32, tag="y")
                # (z - mean) * rstd  via  z*rstd + (-mean*rstd)
                nc.scalar.activation(y[:], z[:], AF.Identity,
                                     bias=negm2[:], scale=rstd[:])
                nc.vector.tensor_tensor(y[:], y[:], gamma_rep[:], ALU.mult)
                nc.vector.tensor_tensor(y[:], y[:], beta_rep[:], ALU.add)
                nc.sync.dma_start(out_r[:, ic, :], y[:])

    nc.compile()
    return nc


def _get_program():
    if "nc" not in _CACHE:
        _CACHE["nc"] = _build_program()
    return _CACHE["nc"]


def make_in_maps(inputs):
    q = np.ascontiguousarray(np.asarray(inputs["q"], dtype=np.float32))
    kv = np.ascontiguousarray(np.asarray(inputs["kv"], dtype=np.float32))
    Wq = np.ascontiguousarray(np.asarray(inputs["Wq"], dtype=np.float32))
    Wkv = np.asarray(inputs["Wkv"], dtype=np.float32)
    Wk = np.ascontiguousarray(Wkv[:, :D])
    Wv = np.ascontiguousarray(Wkv[:, D:])
    Wfc = np.ascontiguousarray(np.asarray(inputs["Wfc"], dtype=np.float32))
    bfc = np.ascontiguousarray(np.asarray(inputs["bfc"], dtype=np.float32))
    gamma = np.ascontiguousarray(np.asarray(inputs["gamma"], dtype=np.float32))
    beta = np.ascontiguousarray(np.asarray(inputs["beta"], dtype=np.float32))

    kvT = [np.ascontiguousarray(kv[b].T) for b in range(B)]
    in_maps = []
    for c in range(8):
        b, r = divmod(c, 2)
        qT = np.ascontiguousarray(q[b, r * ROWS:(r + 1) * ROWS, :].T)
        in_maps.append({
            "qT": qT, "kvT": kvT[b],
            "Wq": Wq, "Wk": Wk, "Wv": Wv, "Wfc": Wfc,
            "bfc": bfc, "gamma": gamma, "beta": beta,
        })
    return in_maps


def kernel(**inputs):
    from concourse.bass_utils import run_bass_kernel_spmd

    nc = _get_program()
    in_maps = make_in_maps(inputs)
    res = run_bass_kernel_spmd(nc, in_maps, core_ids=list(range(8)))
    out = np.empty((B, NQ, D), dtype=np.float32)
    for c in range(8):
        b, r = divmod(c, 2)
        out[b, r * ROWS:(r + 1) * ROWS, :] = res.results[c]["out"]
    return out
